# revision 31
# baseline (speedup 1.0000x reference)
"""AudioSNN forward pass on 8 Trainium2 NeuronCores (pure data parallel).

Host side: weight re-layout + padding (numpy). Device side: Bass/Tile kernel
per core over a 128-batch shard.

Structure per core:
  conv1 (9-tap block-diag matmul, 1 round) -> spike -> pool1 -> scatter into
  dx-replicated P1 -> conv2 (K=96, 3 dy rounds, paired PSUM (r0,r2)/(r1,r3))
  -> spike -> pool2 -> c2 counts (bf16) -> fc1 (bf16 hi/lo, K=128 rounds)
  -> 25-step LIF chains (DVE + Scalar spike ACTs, GpSimd layer-5) with
  per-step fc2/fc3 matmuls; fc2/fc3 biases folded into shifted membrane
  state (m' = m - bias/(1-beta)) so no per-step bias op is needed.
"""
import os
import sys
import numpy as np

for _p in ("/opt/trn_rl_repo", "/root/.axon_site/_ro/trn_rl_repo"):
    if os.path.isdir(_p) and _p not in sys.path:
        sys.path.insert(0, _p)

import ml_dtypes
from contextlib import ExitStack

import concourse.bass as bass
import concourse.tile as tile
from concourse import mybir, bacc
from concourse.bass_utils import run_bass_kernel_spmd

BF16 = mybir.dt.bfloat16
F32 = mybir.dt.float32
GT = mybir.AluOpType.is_gt
ADD = mybir.AluOpType.add
SUB = mybir.AluOpType.subtract
MUL = mybir.AluOpType.mult
SIGMOID = mybir.ActivationFunctionType.Sigmoid

N_CORES = 8
B = 1024
BL = B // N_CORES        # 128 batch per core
H, W = 64, 32            # conv1 image
HP, WP = H + 3, W + 3    # padded (67, 35)
H2, W2 = 32, 16          # conv2 image (after pool1)
H2P, W2P = H2 + 2, W2 + 2  # 34, 18
NS = 25
BETA = 0.95
SCALE = float(2.0 ** 96)  # sigmoid saturation scale (exact power of two)

BC = 32                  # conv2 batch-chunk
NCHUNK = BL // BC        # 4
P1COLS = H2P * W2P * BC  # 34*18*32 = 19584
XW = H * WP              # 2240: one conv1 window row-block


def build_program():
    nc = bacc.Bacc()

    # ---- inputs (host-preprocessed layouts) ----
    # xrall[16*(chunk*4+gg) .. +128, :]: conv1 window gather, precomputed host-side
    XRALL = nc.declare_dram_parameter("xrall", [16 * 128, XW], BF16, isOutput=False)
    WC1 = nc.declare_dram_parameter("wc1", [128, 128], BF16, isOutput=False)
    WC2 = nc.declare_dram_parameter("wc2", [96, 192], BF16, isOutput=False)
    # vecs cols: 0 b1s, 1 b2s, 2 t2, 3 fc1b_h0, 4 fc1b_h1, 5 bk4, 6 b4s, 7 pad
    VECS = nc.declare_dram_parameter("vecs", [128, 8], F32, isOutput=False)
    VECS16 = nc.declare_dram_parameter("vecs16", [128, 48], F32, isOutput=False)
    FC2W = nc.declare_dram_parameter("fc2w", [128, 512], BF16, isOutput=False)
    FC3W = nc.declare_dram_parameter("fc3w", [128, 32], BF16, isOutput=False)
    FC1H = nc.declare_dram_parameter("fc1h", [128, 16384], BF16, isOutput=False)
    FC1L = nc.declare_dram_parameter("fc1l", [128, 16384], BF16, isOutput=False)

    OUT = nc.declare_dram_parameter("out", [128, NS * 16], F32, isOutput=True)

    dbg = os.environ.get("KERNEL_DEBUG", "")
    dbg_outs = {}
    if dbg:
        dbg_outs["p1"] = nc.declare_dram_parameter("dbg_p1", [128, P1COLS], BF16, isOutput=True)
        dbg_outs["c2"] = nc.declare_dram_parameter("dbg_c2", [128, 8192], BF16, isOutput=True)
        dbg_outs["cur3"] = nc.declare_dram_parameter("dbg_cur3", [128, 256], F32, isOutput=True)

    with tile.TileContext(nc) as tc, ExitStack() as top:
        consts = top.enter_context(tc.tile_pool(name="consts", bufs=1))

        # small consts: wc1+vecs on sync queue (needed first); rest on act queue
        wc1 = consts.tile([128, 128], BF16, tag="wc1")
        nc.sync.dma_start(wc1[:], WC1[:])
        vecs = consts.tile([128, 8], F32, tag="vecs")
        nc.sync.dma_start(vecs[:], VECS[:])
        wc2 = consts.tile([96, 192], BF16, tag="wc2")
        nc.scalar.dma_start(wc2[:], WC2[:])
        vecs16 = consts.tile([128, 48], F32, tag="vecs16")
        nc.scalar.dma_start(vecs16[:], VECS16[:])
        fc2w = consts.tile([128, 512], BF16, tag="fc2w")
        nc.scalar.dma_start(fc2w[:], FC2W[:])
        fc3w = consts.tile([128, 32], BF16, tag="fc3w")
        nc.scalar.dma_start(fc3w[:], FC3W[:])

        b1s = vecs[:, 0:1]
        b2s = vecs[:, 1:2]
        t2v = vecs[:, 2:3]
        bk4 = vecs[:, 5:6]
        b4s = vecs[:, 6:7]
        negscale = vecs[:, 7:8]
        thr5b = vecs16[:, 0:16]
        bk5b = vecs16[:, 16:32]
        beta5b = vecs16[:, 32:48]

        # fc1 weights: prefetch on act queue (2 DMAs), overlaps conv phase
        w1pool = top.enter_context(tc.tile_pool(name="w1pool", bufs=1))
        wfc = w1pool.tile([128, 32768], BF16, tag="wfc")
        nc.scalar.dma_start(wfc[:, 0:16384], FC1H[:])
        nc.scalar.dma_start(wfc[:, 16384:32768], FC1L[:])

        # identity for the PE cur3 transpose
        ident = consts.tile([128, 128], F32, tag="ident")
        import concourse.masks as masks
        masks.make_identity(nc, ident[:])

        # c2 accumulation buffer: [128 = (jpar, ch), 64 r x 128 batch] bf16
        c2buf = top.enter_context(tc.tile_pool(name="c2bufp", bufs=1)).tile(
            [128, 8192], BF16, tag="c2buf")

        # fc1 accumulator lives at top level: its matmuls interleave with the
        # last conv2 chunk (row-block q of c2 completes with quad q)
        fc1ps_pool = top.enter_context(tc.tile_pool(name="fc1ps", bufs=1, space="PSUM"))
        ps3 = fc1ps_pool.tile([128, 256], F32, tag="ps3")

        # ------------- conv1 + spike1 + pool1 + conv2 + spike2 + pool2 -------------
        with ExitStack() as convs:
            p1pool = convs.enter_context(tc.tile_pool(name="p1pool", bufs=1))
            xrep_pool = convs.enter_context(tc.tile_pool(name="xrep", bufs=3))
            s1pool = convs.enter_context(tc.tile_pool(name="s1pool", bufs=2))
            pwpool = convs.enter_context(tc.tile_pool(name="pwpool", bufs=2))
            stgpool = convs.enter_context(tc.tile_pool(name="stgpool", bufs=1))
            c1ps = convs.enter_context(tc.tile_pool(name="c1ps", bufs=1, space="PSUM"))
            c2ps = convs.enter_context(tc.tile_pool(name="c2ps", bufs=4, space="PSUM"))
            s2pool = convs.enter_context(tc.tile_pool(name="s2pool", bufs=4))
            pw2pool = convs.enter_context(tc.tile_pool(name="pw2pool", bufs=4))

            # persistent double-buffered P1; pad rows (h2p = 0, 33) and block-0
            # left-pad column (w = 0) zeroed once.
            p1bufs = [p1pool.tile([128, P1COLS], BF16, tag=f"p1_{i}", name=f"p1_{i}")
                      for i in range(2)]
            for i in range(2):
                pv = p1bufs[i][:].rearrange("p (b h w) -> p b h w", b=BC, h=H2P, w=W2P)
                nc.vector.memset(pv[0:96, :, 0, :], 0.0)
                nc.vector.memset(pv[0:96, :, H2P - 1, :], 0.0)
                nc.vector.memset(pv[0:32, :, :, 0], 0.0)
            # persistent pool1 staging [128 = (4ch + b4), (sub, h2, w2p)]:
            # interior at w2p 0..15, zero pads at 16..17 (left-aligned so the
            # pool1-h write is 4B aligned -> DVE 2x mode).
            stages = [stgpool.tile([128, 2 * H2 * W2P], BF16, tag=f"stg_{i}",
                                   name=f"stg_{i}") for i in range(3)]
            for i in range(3):
                sv = stages[i][:].rearrange("p (s h v) -> p s h v", s=2, h=H2, v=W2P)
                nc.vector.memset(sv[:, :, :, W2:W2P], 0.0)

            # conv1 window gathers: prefetch 2 groups ahead so scatter waits
            # on the sync queue never block the next gather.
            xr_tiles = {}

            def issue_xr(g):
                if g >= 16:
                    return
                t = xrep_pool.tile([128, XW], BF16, tag="xr", name=f"xr_{g}")
                nc.sync.dma_start(t[:], XRALL[128 * g:128 * (g + 1), :])
                xr_tiles[g] = t

            issue_xr(0)
            issue_xr(1)

            c2r = c2buf[:].rearrange("p (r b) -> p r b", b=BL)

            def conv1_unit(chunk, gg):
                p1 = p1bufs[chunk % 2]
                g = chunk * 4 + gg
                xr = xr_tiles.pop(g)
                xrv = xr[:].rearrange("p (h w) -> p h w", h=H, w=WP)
                stg = stages[gg % 3]
                sv = stg[:].rearrange("p (s h v) -> p s h v", s=2, h=H2, v=W2P)

                for sub in range(2):
                    sb = 64 * sub
                    # conv1: 2 psum tiles of [128, 1024], 2 matmuls each
                    s1 = s1pool.tile([128, H * W], BF16, tag="s1",
                                     name=f"s1_{g}_{sub}")
                    # s1 layout (t, h, w2) with t = w parity -> pools run
                    # on contiguous operands (DVE 2x mode)
                    s1o = s1[:].rearrange("p (t h w2) -> p h t w2",
                                          t=2, h=H, w2=W // 2)
                    for half in range(2):
                        ps1 = c1ps.tile([128, 1024], F32, tag="c1ps")
                        for j in range(2):
                            q4 = 2 * half + j
                            nc.tensor.matmul(
                                ps1[:, 512 * j:512 * (j + 1)],
                                wc1[sb:sb + 36, :],
                                xrv[sb:sb + 36, 16 * q4:16 * q4 + 16, 0:W],
                                start=True, stop=True,
                                tile_position=(sb, 0))
                        in_v = ps1[:].rearrange("p (r w2 t) -> p r t w2",
                                                r=32, w2=W // 2, t=2)
                        nc.scalar.activation(
                            s1o[:, 32 * half:32 * (half + 1), :, :], in_v,
                            SIGMOID, bias=b1s, scale=SCALE)

                    # pool1 w-pairs -> pw layout (t2, h2, w2), h = 2h2+t2
                    pw = pwpool.tile([128, H * (W // 2)], BF16, tag="pw",
                                     name=f"pw_{g}_{sub}")
                    s1v = s1[:].rearrange("p (t h2 t2 w2) -> p h2 t2 w2 t",
                                          t=2, h2=H2, t2=2, w2=W // 2)
                    pwv = pw[:].rearrange("p (t2 h2 w2) -> p h2 t2 w2",
                                          t2=2, h2=H2, w2=W // 2)
                    nc.vector.tensor_add(pwv, s1v[:, :, :, :, 0],
                                         s1v[:, :, :, :, 1])
                    # pool1 h-pairs into staging interior (w2p 0..15)
                    nc.vector.tensor_add(sv[:, sub, :, 0:W2],
                                         pwv[:, :, 0, :], pwv[:, :, 1, :])

                # prefetch the gather 2 groups ahead BEFORE this group's
                # scatters hit the sync queue
                issue_xr(g + 2)

                # scatter both subs into P1, one DMA per dx-replica.
                # stg partitions are (4*ch + b4); P1 batch order within a
                # chunk is bp = gg*8 + 2*b4 + sub (host unpermutes output).
                # src runs iterate ((ch, b4) partitions, sub); dst runs
                # iterate (ch, (b4, sub)) -- identical zip order.
                CELL = H2 * W2P  # 576
                BLK = H2P * W2P  # 612
                src = bass.AP(stg.tensor, stg.offset,
                              [[2 * CELL, 128], [CELL, 2], [1, CELL]])
                for rep in range(3):
                    dst = bass.AP(p1.tensor,
                                  p1.offset + rep * 32 * P1COLS
                                  + gg * 8 * BLK + W2P + 1 - rep,
                                  [[P1COLS, 32], [BLK, 8], [1, CELL]])
                    nc.sync.dma_start(dst, src)

            def conv2_quad(chunk, quad):
                # conv2: K=96 (3 dx-replicas), 3 dy rounds, paired rows
                p1 = p1bufs[chunk % 2]
                p1v = p1[:].rearrange("p (b h w) -> p b h w", b=BC, h=H2P, w=W2P)
                pss = [c2ps.tile([128, W2 * BC], F32, tag="c2ps",
                                 name=f"ps2_{chunk}_{quad}_{i}") for i in range(2)]
                for i in range(2):       # psA: rows 4q+0/4q+2; psB: +1/+3
                    for dy in range(3):
                        wblk = wc2[0:96, 64 * dy:64 * (dy + 1)]
                        for colpos, roff in ((0, 0), (64, 2)):
                            hrow = 4 * quad + i + roff
                            nc.tensor.matmul(
                                pss[i][colpos:colpos + 64, :], wblk,
                                p1v[0:96, :, hrow + dy, 0:W2],
                                start=(dy == 0), stop=(dy == 2),
                                tile_position=(0, colpos))
                # spikes: psA via Scalar ACT, psB via DVE GT; s2 layout
                # (t, b, w2) with t = w parity so pool2-w is contiguous
                s2a = s2pool.tile([128, W2 * BC], BF16, tag="s2",
                                  name=f"s2a_{chunk}_{quad}")
                s2b = s2pool.tile([128, W2 * BC], BF16, tag="s2",
                                  name=f"s2b_{chunk}_{quad}")
                for ps_, s2 in ((pss[0], s2a), (pss[1], s2b)):
                    in_v = ps_[:].rearrange("p (b w2 t) -> p b t w2",
                                            b=BC, w2=W2 // 2, t=2)
                    out_v = s2[:].rearrange("p (t b w2) -> p b t w2",
                                            t=2, b=BC, w2=W2 // 2)
                    if s2 is s2a:
                        nc.scalar.activation(out_v, in_v, SIGMOID,
                                             bias=b2s, scale=SCALE)
                    else:
                        nc.vector.tensor_scalar(out_v, in_v, t2v, None, op0=GT)
                # pool2 w-pairs: A on DVE, B on GpSimd (contiguous halves)
                pw2 = []
                for nm, s2 in (("a", s2a), ("b", s2b)):
                    p_ = pw2pool.tile([128, (W2 // 2) * BC], BF16, tag="pw2",
                                      name=f"pw2{nm}_{chunk}_{quad}")
                    half = (W2 // 2) * BC
                    eng = nc.vector if nm == "a" else nc.gpsimd
                    eng.tensor_add(p_[:], s2[:, 0:half], s2[:, half:2 * half])
                    pw2.append(p_)
                # pool2 h-pairs -> c2buf: partitions 0:64 j=2q, 64:128 j=2q+1
                dst = c2r[:, quad * 8:(quad + 1) * 8,
                          chunk * BC:(chunk + 1) * BC].transpose([0, 2, 1])
                pa = pw2[0][:].rearrange("p (b w) -> p b w", b=BC)
                pb = pw2[1][:].rearrange("p (b w) -> p b w", b=BC)
                nc.vector.tensor_add(dst, pa, pb)

            # software pipeline: conv1(k) interleaves with conv2(k-1) so no
            # engine queue is head-of-line blocked at chunk boundaries.
            # conv1 units front-loaded; fc1 matmuls chase the last conv2 chunk
            # (r-block q of c2 is complete once quad q of chunk 3 is done).
            for chunk in range(NCHUNK + 1):
                for step in range(8):
                    if chunk < NCHUNK and step < 4:
                        conv1_unit(chunk, step)
                    if chunk >= 1:
                        conv2_quad(chunk - 1, step)
                        if chunk == NCHUNK:
                            q = step
                            for r in range(8 * q, 8 * q + 8):
                                for j in range(2):
                                    col = (r * 2 + j) * 256
                                    nc.tensor.matmul(
                                        ps3[:], c2r[:, r, :],
                                        wfc[:, col:col + 256],
                                        start=(r == 0 and j == 0),
                                        stop=(r == 63 and j == 1))
                if dbg and chunk == 0:
                    nc.sync.dma_start(dbg_outs["p1"][:], p1bufs[0][:])

        if dbg:
            nc.sync.dma_start(dbg_outs["c2"][:], c2buf[:])

        # ---------------- fc1 (bf16 hi/lo) + LIF ----------------
        with ExitStack() as fcs:
            lifc = fcs.enter_context(tc.tile_pool(name="lifc", bufs=1))
            cur3c = lifc.tile([128, 256], F32, tag="cur3c")

            # fc1 matmuls already accumulated into ps3 during the last conv2
            # chunk. Scale 0.25, stage to SBUF, transpose via PE (reusing the
            # ps3 bank as transpose target), add bias.
            c3t = lifc.tile([128, 256], F32, tag="c3t")
            nc.scalar.mul(c3t[:], ps3[:], 0.25)
            for h in range(2):
                psT = ps3[:, 128 * h:128 * (h + 1)]
                nc.tensor.transpose(psT, c3t[:, h * BL:(h + 1) * BL], ident[:])
                nc.vector.tensor_scalar(cur3c[:, h * BL:(h + 1) * BL],
                                        psT, 1.0, vecs[:, 3 + h:4 + h],
                                        op0=MUL, op1=ADD)
            if dbg:
                nc.sync.dma_start(dbg_outs["cur3"][:], cur3c[:])

            # LIF state + buffers
            lifps = fcs.enter_context(tc.tile_pool(name="lifps", bufs=1, space="PSUM"))
            p4tiles = [lifps.tile([128, 512], F32, tag=f"p4_{i}", name=f"p4_{i}")
                       for i in range(6)]
            p5 = lifps.tile([128, 512], F32, tag="p5")

            m3 = lifc.tile([128, 256], F32, tag="m3")
            t3 = lifc.tile([128, 256], F32, tag="t3")
            spk3buf = lifc.tile([128, NS * 256], BF16, tag="spk3buf")
            m4 = lifc.tile([128, 128], F32, tag="m4")
            t4 = lifc.tile([128, 128], F32, tag="t4")
            spk4buf = lifc.tile([128, NS * 128], BF16, tag="spk4buf")
            m5 = lifc.tile([128, 16], F32, tag="m5")
            t5 = lifc.tile([128, 16], F32, tag="t5")
            cur5buf = lifc.tile([128, NS * 16], F32, tag="cur5buf")
            outstage = lifc.tile([128, NS * 16], F32, tag="outstage")

            for st in range(NS):
                s3 = spk3buf[:, st * 256:(st + 1) * 256]
                # layer 3 (units 0..255 on (h, b) free layout); cur3 constant
                if st == 0:
                    nc.scalar.activation(s3, cur3c[:], SIGMOID,
                                         bias=negscale, scale=SCALE)
                else:
                    prev_m = cur3c if st == 1 else m3
                    nc.vector.scalar_tensor_tensor(t3[:], prev_m[:], BETA,
                                                   cur3c[:], op0=MUL, op1=ADD)
                    nc.vector.tensor_sub(m3[:], t3[:],
                                         spk3buf[:, (st - 1) * 256:st * 256])
                    nc.scalar.activation(s3, m3[:], SIGMOID,
                                         bias=negscale, scale=SCALE)
                # fc2 -> p4[st] (bias folded into shifted membrane state)
                sl = st % 24
                p4 = p4tiles[sl // 4][:, (sl % 4) * 128:(sl % 4 + 1) * 128]
                nc.tensor.matmul(p4, fc2w[:, 0:128], s3[:, 0:128],
                                 start=True, stop=False)
                nc.tensor.matmul(p4, fc2w[:, 128:256], s3[:, 128:256],
                                 start=False, stop=False)
                nc.tensor.matmul(p4, fc2w[:, 256:384], s3[:, 0:128],
                                 start=False, stop=False)
                nc.tensor.matmul(p4, fc2w[:, 384:512], s3[:, 128:256],
                                 start=False, stop=True)
                # layer 4: m' = m - 20*fc2_b; threshold thr4' = 1 - 20*fc2_b
                s4 = spk4buf[:, st * 128:(st + 1) * 128]
                if st == 0:
                    nc.vector.tensor_scalar(m4[:], p4, bk4, None, op0=SUB)
                else:
                    nc.vector.scalar_tensor_tensor(t4[:], m4[:], BETA, p4,
                                                   op0=MUL, op1=ADD)
                    nc.vector.tensor_sub(m4[:], t4[:],
                                         spk4buf[:, (st - 1) * 128:st * 128])
                nc.scalar.activation(s4, m4[:], SIGMOID, bias=b4s, scale=SCALE)
                # fc3 transposed: out[b, u5] = spk4.T @ w3T(hi+lo)
                p5s = p5[:, st * 16:(st + 1) * 16]
                nc.tensor.matmul(p5s, s4, fc3w[:, 0:16], start=True, stop=False)
                nc.tensor.matmul(p5s, s4, fc3w[:, 16:32], start=False, stop=True)
                # layer 5 on GpSimd (no PSUM access there: stage via Scalar)
                c5 = cur5buf[:, st * 16:(st + 1) * 16]
                nc.scalar.copy(c5, p5s)
                s5 = outstage[:, st * 16:(st + 1) * 16]
                if st == 0:
                    nc.gpsimd.tensor_sub(m5[:], c5, bk5b)
                else:
                    nc.gpsimd.tensor_mul(t5[:], m5[:], beta5b)
                    nc.gpsimd.tensor_add(t5[:], t5[:], c5)
                    nc.gpsimd.tensor_sub(m5[:], t5[:],
                                         outstage[:, (st - 1) * 16:st * 16])
                nc.vector.tensor_tensor(s5, m5[:], thr5b, op=GT)

            nc.sync.dma_start(OUT[:], outstage[:])

    nc.compile()
    return nc


def _prep_inputs(x, conv1_w, conv1_b, conv2_w, conv2_b, fc1_w, fc1_b,
                 fc2_w, fc2_b, fc3_w, fc3_b):
    """Host-side preprocessing -> list of 8 per-core input dicts."""
    bf = ml_dtypes.bfloat16

    # conv1 weights: 2 replicas of block-diag [36 = (3dy x 3dx) x 4b, 128];
    # out partition m = 4*ch + b4 (so pool1 staging scatters with
    # partition-aligned DMA strides)
    wc1 = np.zeros((128, 128), np.float32)
    w1 = conv1_w.reshape(32, 3, 3)  # [c, dy, dx]
    for sub in range(2):
        for dy in range(3):
            for dx in range(3):
                k = 3 * dy + dx
                for b4 in range(4):
                    wc1[64 * sub + 4 * k + b4, 4 * np.arange(32) + b4] = w1[:, dy, dx]
    wc1 = wc1.astype(bf)

    thr1 = (1.0 - conv1_b).astype(np.float64)            # [32]
    b1s = np.repeat(-thr1 * SCALE, 4).astype(np.float32)  # [128] = m = 4ch+b4

    # conv2 weights: lhsT [96 = 3dx x 32ch_in, 3dy x 64 ch_out]
    wc2 = np.zeros((96, 192), np.float32)
    for dx in range(3):
        for dy in range(3):
            wc2[32 * dx:32 * (dx + 1), 64 * dy:64 * (dy + 1)] = conv2_w[:, :, dy, dx].T
    wc2 = wc2.astype(bf)
    thr2 = (4.0 * (1.0 - conv2_b)).astype(np.float64)  # [64]
    t2 = np.tile(thr2, 2)                              # [128]
    b2s = (-t2 * SCALE).astype(np.float32)

    # fc1 weights (flipped matmul: weights are the moving operand):
    # [128 = (jpar, ch) feat partitions, (r=(h2,w'), j=hi/lo, u=(h,m))] bf16
    # feat = ch*128 + h'*8 + w', h' = 2*h2 + jpar
    W4 = fc1_w.reshape(2, 128, 64, 8, 2, 8)  # [h, m, ch, h2, jpar, w']
    fc1t = np.ascontiguousarray(W4.transpose(4, 2, 0, 3, 5, 1))  # [jp,ch,h,h2,w',m]
    A = fc1t.reshape(128, 2, 64, 128)        # [p, h, r, m]
    hi = A.astype(bf)
    lo = (A - hi.astype(np.float32)).astype(bf)
    st = np.stack([hi, lo], axis=2)          # [p, h, j, r, m]
    fc1n = np.ascontiguousarray(st.transpose(0, 3, 2, 1, 4)).reshape(128, 32768)
    fc1h = fc1n[:, 0:16384]
    fc1l = fc1n[:, 16384:32768]
    fc1bt = np.ascontiguousarray(fc1_b.reshape(2, 128).T).astype(np.float32)

    # fc2: lhsT [u3, u4] hi/lo, chunk-major columns
    l2 = np.ascontiguousarray(fc2_w.T).astype(np.float32)   # [256 u3, 128 u4]
    l2a_full = l2.astype(bf)
    l2b_full = (l2 - l2a_full.astype(np.float32)).astype(bf)
    def chunked(a):  # [256, 128] -> [128, 256] with chunk-major cols
        return np.ascontiguousarray(a.reshape(2, 128, 128).transpose(1, 0, 2).reshape(128, 256))
    fc2w_all = np.concatenate([chunked(l2a_full), chunked(l2b_full)], axis=1)  # [128, 512]

    # fc3 transposed: [128 u4, 16] hi/lo
    l3 = np.zeros((128, 16), np.float32)
    l3[:, 0:10] = fc3_w.T
    l3a = l3.astype(bf)
    l3b = (l3 - l3a.astype(np.float32)).astype(bf)
    fc3w_all = np.concatenate([l3a, l3b], axis=1)  # [128, 32]

    # LIF bias folds: m' = m - K, K = bias/(1-beta) = 20*bias
    b2f = fc2_b.astype(np.float64)
    bk4 = (BETA * 20.0 * b2f).astype(np.float32)               # [128]
    b4s = (-(1.0 - 20.0 * b2f) * SCALE).astype(np.float32)     # [128]
    b3f = np.zeros(16, np.float64)
    b3f[0:10] = fc3_b
    thr5 = (1.0 - 20.0 * b3f).astype(np.float32)               # [16]
    bk5 = (BETA * 20.0 * b3f).astype(np.float32)

    vecs = np.zeros((128, 8), np.float32)
    vecs[:, 0] = b1s
    vecs[:, 1] = b2s
    vecs[:, 2] = t2.astype(np.float32)
    vecs[:, 3:5] = fc1bt
    vecs[:, 5] = bk4
    vecs[:, 6] = b4s
    vecs[:, 7] = -SCALE
    vecs16 = np.zeros((128, 48), np.float32)
    vecs16[:, 0:16] = thr5[None, :]
    vecs16[:, 16:32] = bk5[None, :]
    vecs16[:, 32:48] = BETA

    common = dict(wc1=wc1, wc2=wc2, vecs=vecs, vecs16=vecs16,
                  fc2w=fc2w_all.astype(bf), fc3w=fc3w_all.astype(bf),
                  fc1h=fc1h, fc1l=fc1l)

    # conv1 gather: xrall[g = chunk*4+gg, 64*sub + 4*(3dy+dx) + b4, :] =
    #   padded_x[g*8 + 4*sub + b4, dy*WP + dx : +2240]
    xs = x.reshape(B, H, W).astype(bf)
    # row/offset index tables (same for every core)
    rows = np.zeros((2, 3, 3, 4), np.int64)
    boff = np.zeros((2, 3, 3, 4), np.int64)
    woff = np.zeros((2, 3, 3, 4), np.int64)
    for sub in range(2):
        for dy in range(3):
            for dx in range(3):
                for b4 in range(4):
                    rows[sub, dy, dx, b4] = 64 * sub + 4 * (3 * dy + dx) + b4
                    boff[sub, dy, dx, b4] = 4 * sub + b4
                    woff[sub, dy, dx, b4] = dy * WP + dx
    rows = rows.ravel()
    boff = boff.ravel()
    woff = woff.ravel()

    in_maps = []
    for c in range(N_CORES):
        xc = xs[c * BL:(c + 1) * BL]
        xp = np.zeros((BL + 1, HP * WP), bf)
        xp.reshape(BL + 1, HP, WP)[:BL, 1:H + 1, 1:W + 1] = xc
        win = np.lib.stride_tricks.sliding_window_view(xp, XW, axis=1)  # [BL+1, ., XW]
        xrall = np.zeros((16, 128, XW), bf)
        for g in range(16):
            xrall[g, rows] = win[g * 8 + boff, woff]
        m = dict(common)
        m["xrall"] = xrall.reshape(16 * 128, XW)
        in_maps.append(m)
    return in_maps


def batch_perm():
    """perm[bp] = actual batch index (within a core) for device batch slot bp.

    Device slots are ordered (chunk, gg, b4, sub); actual batches are
    (chunk, gg, sub, b4).
    """
    perm = np.zeros(BL, np.int64)
    for chunk in range(4):
        for gg in range(4):
            for b4 in range(4):
                for sub in range(2):
                    bp = ((chunk * 4 + gg) * 4 + b4) * 2 + sub
                    perm[bp] = (chunk * 4 + gg) * 8 + 4 * sub + b4
    return perm


_NC_CACHE = {}


def _get_nc():
    if "nc" not in _NC_CACHE:
        _NC_CACHE["nc"] = build_program()
    return _NC_CACHE["nc"]


def kernel(**inputs):
    nc = _get_nc()
    in_maps = _prep_inputs(**inputs)
    res = run_bass_kernel_spmd(nc, in_maps, core_ids=list(range(N_CORES)))
    perm = batch_perm()
    outs = []
    for c in range(N_CORES):
        o = res.results[c]["out"]            # [128, NS*16], batch rows permuted
        o = o.reshape(BL, NS, 16)[:, :, 0:10]
        u = np.empty_like(o)
        u[perm] = o
        outs.append(u.transpose(1, 0, 2))    # [NS, BL, 10]
    return np.concatenate(outs, axis=1).astype(np.float32)  # [NS, B, 10]
